# revision 42
# baseline (speedup 1.0000x reference)
"""TRN2 Bass kernel for nn_CSWinB (CSWin attention block), 8-core SPMD.

Sharding: core c = (batch b=c//2, branch br=c%2). Each core computes its
batch's full embed+LN, its branch's qkv slice, 8 windows of stripe attention
with LePE, and partial (half-contraction) out/vout GEMMs; host sums the two
partials per batch.

Device layouts are channel-major [C, tokens] with a per-branch token
permutation making windows contiguous 512-token blocks and unifying both
branches to a [64, 8] window image (branch 1's image is transposed; its conv
taps are transposed on the host to match).

Performance structure (610us -> ~365us on-device):
- Software-pipelined window schedule: window w+1's embed/LN-stats/qkv is
  interleaved between window w's attention head pairs, so the PE never
  stalls on the serial stats chain (PE ssq <- ACT sq <- DVE copy <- embed).
- Softmax normalize via the ones-column trick + DVE reciprocal_approx_fast
  (base-partition-0 operands only: the custom DVE op corrupts on
  partition-offset APs, hence sums in pv[0:32] / o in pv[32:64]).
- exp written as fp8e4 and v_aug stored fp8e4; PV uses fp8 DoubleRow
  matmuls (two jt k-tiles accumulated per instruction, ~2x PE rate).
- Score matmuls j2-interleaved across head pairs so consecutive PE
  instructions hit different tile_position row bands (weight loads overlap).
- LN rank-2 correction folded away: x2s = x2t*a - (mu*a) via two PE
  broadcasts + DVE mul/sub; W^T b1 bias applied in the PSUM->SBUF copies
  (ACT Identity-with-bias for q/k/v).
- LePE split: dy=0 taps as PE diagonal matmuls, dy=+-1 taps as DVE
  scalar_tensor_tensor accumulation interleaved between head pairs.
"""
import sys
sys.path.insert(0, '/opt/trn_rl_repo')
from contextlib import ExitStack

import numpy as np
import ml_dtypes

import concourse.bass as bass
import concourse.tile as tile
import concourse.mybir as mybir
import concourse.bass_isa as bass_isa
from concourse import bacc
from concourse.bass_utils import run_bass_kernel_spmd

B, DIM = 4, 256
L = 4096
C2, CB, NH, HD = 512, 256, 8, 32
SCALE = HD ** -0.5
EPS = 1e-5
NWIN, WIN = 8, 512
R, C = 64, 8            # unified window image
CP = C + 2              # zero-padded columns
BLK = R * CP            # 640 padded tokens per window

f32 = mybir.dt.float32
f32r = mybir.dt.float32r
bf16 = mybir.dt.bfloat16
fp8 = mybir.dt.float8e4
DR = mybir.MatmulPerfMode.DoubleRow
AF = mybir.ActivationFunctionType
ALU = mybir.AluOpType

# taps ordered center-first so the center matmul (full region) opens the
# PSUM accumulation group and initializes every element
TAPS = [(0, 0)] + [(dy, dx) for dy in (-1, 0, 1) for dx in (-1, 0, 1)
                   if (dy, dx) != (0, 0)]


def _ap(t, off, pattern):
    return bass.AP(tensor=t.tensor, offset=t.offset + off,
                   ap=[t.ap[0]] + pattern)


def build_nc():
    nc = bacc.Bacc("TRN2", target_bir_lowering=False, debug=False)
    xb = nc.dram_tensor("xb", [256, L], f32r, kind="ExternalInput").ap()
    wemb = nc.dram_tensor("wemb", [256, 513], f32r, kind="ExternalInput").ap()
    wqkv = nc.dram_tensor("wqkv", [512, 768], f32r, kind="ExternalInput").ap()
    woutb = nc.dram_tensor("woutb", [256, 256], f32r, kind="ExternalInput").ap()
    wy = nc.dram_tensor("wy", [256, 256], f32r, kind="ExternalInput").ap()
    w2 = nc.dram_tensor("w2", [256, 256], f32r, kind="ExternalInput").ap()
    ldiag = nc.dram_tensor("ldiag", [128, 768], bf16, kind="ExternalInput").ap()
    w18d = nc.dram_tensor("w18", [128, 18], f32, kind="ExternalInput").ap()
    ident = nc.dram_tensor("ident", [128, 128], f32r, kind="ExternalInput").ap()
    onesr = nc.dram_tensor("onesr", [1, 128], f32r, kind="ExternalInput").ap()
    onesc = nc.dram_tensor("onesc", [128, 1], f32r, kind="ExternalInput").ap()
    ones256 = nc.dram_tensor("ones256", [128, 256], f32r, kind="ExternalInput").ap()
    wbiasd = nc.dram_tensor("wbias", [128, 6], f32, kind="ExternalInput").ap()
    cb2d = nc.dram_tensor("cb2", [128, 2], f32, kind="ExternalInput").ap()
    b2cd = nc.dram_tensor("b2c", [128, 2], f32, kind="ExternalInput").ap()
    outp = nc.dram_tensor("outp", [256, L], f32, kind="ExternalOutput").ap()
    voutp = nc.dram_tensor("voutp", [256, L], f32, kind="ExternalOutput").ap()

    with tile.TileContext(nc) as tc, ExitStack() as ctx:
        const = ctx.enter_context(tc.tile_pool(name="const", bufs=1))
        big = ctx.enter_context(tc.tile_pool(name="big", bufs=1))
        dram = ctx.enter_context(tc.tile_pool(name="dram", bufs=1, space="DRAM"))

        # ---------- constants ----------
        wemb_sb = []
        for k in range(2):
            t = const.tile([128, 513], f32r, tag=f"wemb{k}")
            eng = nc.sync if k == 0 else nc.gpsimd
            eng.dma_start(t[:], wemb[128 * k:128 * (k + 1), :])
            wemb_sb.append(t)
        wqkv_sb = []
        for k in range(4):
            t = const.tile([128, 768], f32r, tag=f"wqkv{k}")
            nc.gpsimd.dma_start(t[:], wqkv[128 * k:128 * (k + 1), :])
            wqkv_sb.append(t)
        woutb_sb, wy_sb, w2_sb = [], [], []
        for k in range(2):
            t = const.tile([128, 256], f32r, tag=f"wob{k}")
            nc.gpsimd.dma_start(t[:], woutb[128 * k:128 * (k + 1), :])
            woutb_sb.append(t)
        for k in range(2):
            t = const.tile([128, 256], f32r, tag=f"wy{k}")
            nc.gpsimd.dma_start(t[:], wy[128 * k:128 * (k + 1), :])
            wy_sb.append(t)
        for k in range(2):
            t = const.tile([128, 256], f32r, tag=f"w2{k}")
            nc.gpsimd.dma_start(t[:], w2[128 * k:128 * (k + 1), :])
            w2_sb.append(t)
        onesr_sb = const.tile([1, 128], f32r, tag="onesr")
        nc.gpsimd.dma_start(onesr_sb[:], onesr[:])
        onesc_sb = const.tile([128, 1], f32r, tag="onesc")
        nc.gpsimd.dma_start(onesc_sb[:], onesc[:])
        ones256_sb = const.tile([128, 256], f32r, tag="ones256")
        nc.gpsimd.dma_start(ones256_sb[:], ones256[:])
        wbias_sb = const.tile([128, 6], f32, tag="wbias")
        nc.gpsimd.dma_start(wbias_sb[:], wbiasd[:])
        cb2_sb = const.tile([128, 2], f32, tag="cb2")
        nc.gpsimd.dma_start(cb2_sb[:], cb2d[:])
        b2c_sb = const.tile([128, 2], f32, tag="b2c")
        nc.gpsimd.dma_start(b2c_sb[:], b2cd[:])
        ldiag_sb = const.tile([128, 768], bf16, tag="ldiag")
        nc.gpsimd.dma_start(ldiag_sb[:], ldiag[:])
        w18_sb = const.tile([128, 18], f32, tag="w18")
        nc.gpsimd.dma_start(w18_sb[:], w18d[:])

        # ---------- persistent activations ----------
        va_sb = [big.tile([128, 2048], fp8, name=f"vasb{i}") for i in range(2)]
        ident_sb = const.tile([128, 128], f32r, tag="ident")
        nc.gpsimd.dma_start(ident_sb[:], ident[:])

        for i in range(2):
            # ones columns at 512*jt + 64h + 32..64 for each jt, head h
            for jt in range(4):
                dst = _ap(va_sb[i], 512 * jt, [[64, 8], [1, 32]])
                srcv = ones256_sb[:].rearrange("p (a b) -> p a b", a=8)
                nc.vector.tensor_copy(dst, srcv)

        # fused per-window pipeline: embed/LN/qkv for t-tile w, then window-w
        # attention; PE work of stage A overlaps ACT-bound exp of stage B.
        pools = {}
        for nm, bufs, space in [("xbp", 2, None), ("x2tp", 5, None), ("sqp", 2, None),
                                ("x2sp", 5, None), ("smp", 2, None), ("qkvp", 3, None),
                                ("evp", 2, None), ("vpw", 3, None), ("exp_", 3, None),
                                ("recp", 4, None), ("otwp", 2, None), ("y0wp", 2, None),
                                ("outfp", 2, None), ("pE", 2, "PSUM"), ("pST", 2, "PSUM"), ("pM", 2, "PSUM")]:
            kw = dict(name=nm, bufs=bufs)
            if space:
                kw["space"] = space
            pools[nm] = ctx.enter_context(tc.tile_pool(**kw))
        xbp, x2tp, sqp, x2sp, smp, qkvp, evp = (pools[k] for k in
            ("xbp", "x2tp", "sqp", "x2sp", "smp", "qkvp", "evp"))
        vpwP, exP, recP, otwP, y0wP, outfP = (pools[k] for k in
            ("vpw", "exp_", "recp", "otwp", "y0wp", "outfp"))
        pE, pST, pM = (pools[k] for k in ("pE", "pST", "pM"))
        # ---- software-pipelined window schedule ----------------------------
        # A-phase of window w+1 (embed/stats/qkv) is interleaved between the
        # attention heads of window w so the PE never stalls on the serial
        # LN-stats chain (PE ssq <- ACT sq <- DVE copy <- PE embed).

        def emit_A1(w):
            sl = slice(w * 512, (w + 1) * 512)
            s = {"sl": sl}
            xb_t = []
            for k in range(2):
                t = xbp.tile([128, 512], f32r, tag=f"xb{k}")
                nc.sync.dma_start(t[:], xb[128 * k:128 * (k + 1), sl])
                xb_t.append(t)
            x2t = []
            for ct in range(4):
                ps = pE.tile([128, 512], f32, tag="pe")
                for k in range(2):
                    nc.tensor.matmul(ps[:],
                                     wemb_sb[k][:, ct * 128:(ct + 1) * 128],
                                     xb_t[k][:], start=(k == 0), stop=(k == 1))
                t = x2tp.tile([128, 512], f32r, tag="x2t")
                nc.vector.tensor_copy(t[:], ps[:])
                x2t.append(t)
            mu_ps = pE.tile([1, 512], f32, tag="pe")
            for k in range(2):
                nc.tensor.matmul(mu_ps[:], wemb_sb[k][:, 512:513], xb_t[k][:],
                                 start=(k == 0), stop=(k == 1))
            mu_sb = smp.tile([1, 512], f32, tag="mu")
            nc.vector.tensor_copy(mu_sb[:], mu_ps[:])
            y0w = []
            for ct2 in range(2):
                ps = pE.tile([128, 512], f32, tag="pe")
                for k in range(2):
                    nc.tensor.matmul(ps[:],
                                     wy_sb[k][:, ct2 * 128:(ct2 + 1) * 128],
                                     xb_t[k][:], start=(k == 0), stop=(k == 1))
                t = y0wP.tile([128, 512], f32, tag=f"y0w{ct2}")
                nc.scalar.copy(t[:], ps[:])
                y0w.append(t)
            sq_t = []
            for ct in range(4):
                sq = sqp.tile([128, 512], f32r, tag="sq", bufs=4)
                nc.scalar.square(sq[:], x2t[ct][:])
                sq_t.append(sq)
            mu2 = smp.tile([1, 512], f32, tag="mu2")
            nc.scalar.square(mu2[:], mu_sb[:])
            s.update(xb_t=xb_t, x2t=x2t, mu_sb=mu_sb, y0w=y0w, sq=sq_t, mu2=mu2)
            return s

        def emit_stats(s):
            ssq_ps = pE.tile([1, 512], f32, tag="pe")
            for ct in range(4):
                nc.tensor.matmul(ssq_ps[:], onesc_sb[:], s["sq"][ct][:],
                                 start=(ct == 0), stop=(ct == 3),
                                 skip_group_check=(ct not in (0, 3)))
            var0 = smp.tile([1, 512], f32, tag="var0")
            nc.vector.tensor_scalar(var0[:], ssq_ps[:], 1.0 / C2, EPS,
                                    op0=ALU.mult, op1=ALU.add)
            var = smp.tile([1, 512], f32, tag="var")
            nc.vector.scalar_tensor_tensor(var[:], s["mu2"][:], -1.0, var0[:],
                                           op0=ALU.mult, op1=ALU.add)
            rvar = smp.tile([1, 512], f32, tag="rvar")
            nc.vector.reciprocal_approx_fast(rvar[:], var[:])
            a_sb = smp.tile([1, 512], f32r, tag="a")
            nc.scalar.sqrt(a_sb[:], rvar[:])
            ma_t = smp.tile([1, 512], f32r, tag="maug", bufs=2)
            nc.vector.tensor_mul(ma_t[:], s["mu_sb"][:], a_sb[:])
            s.update(a_sb=a_sb, ma=ma_t)

        def emit_x2s(s):
            abc_ps = pE.tile([128, 512], f32, tag="pe")
            nc.tensor.matmul(abc_ps[:], onesr_sb[:], s["a_sb"][:],
                             start=True, stop=True)
            mab_ps = pE.tile([128, 512], f32, tag="pe")
            nc.tensor.matmul(mab_ps[:], onesr_sb[:], s["ma"][:],
                             start=True, stop=True)
            x2s = []
            for ct in range(4):
                t = x2sp.tile([128, 512], f32r, tag="x2s")
                nc.vector.tensor_mul(t[:], s["x2t"][ct][:], abc_ps[:])
                nc.vector.tensor_tensor(t[:], t[:], mab_ps[:], op=ALU.subtract)
                x2s.append(t)
            s["x2s"] = x2s

        def emit_qk(s):
            qt, kt = [], []
            for ot in range(4):
                ps = pE.tile([128, 512], f32, tag="pe")
                for k in range(4):
                    nc.tensor.matmul(ps[:],
                                     wqkv_sb[k][:, ot * 128:(ot + 1) * 128],
                                     s["x2s"][k][:], start=(k == 0), stop=(k == 3))
                t = qkvp.tile([128, 512], f32r, tag=f"qk{ot}")
                if ot < 2:
                    nc.scalar.activation(t[:], ps[:], AF.Identity,
                                         bias=wbias_sb[:, ot:ot + 1])
                    qt.append(t)
                else:
                    nc.scalar.activation(t[:], ps[:], AF.Identity,
                                         bias=wbias_sb[:, ot:ot + 1])
                    kt.append(t)
            s.update(qt=qt, kt=kt)

        def emit_v(s, w):
            vt, vpadw = [], []
            for ct in range(2):
                ps = pE.tile([128, 512], f32, tag="pe")
                for k in range(4):
                    nc.tensor.matmul(ps[:],
                                     wqkv_sb[k][:, 512 + ct * 128:512 + (ct + 1) * 128],
                                     s["x2s"][k][:], start=(k == 0), stop=(k == 3))
                t = qkvp.tile([128, 512], f32r, tag=f"v{ct}")
                nc.scalar.activation(t[:], ps[:], AF.Identity,
                                     bias=wbias_sb[:, 4 + ct:5 + ct])
                vt.append(t)
                tp = vpwP.tile([128, BLK], bf16, tag=f"vpw{ct}")
                nc.gpsimd.memset(tp[:], 0.0)
                dst = _ap(tp, 1, [[CP, R], [1, C]])
                nc.vector.tensor_copy(dst, t[:].rearrange("p (r c) -> p r c", r=R))
                vpadw.append(tp)
            for ct2 in range(2):
                ps = pE.tile([128, 512], f32, tag="pe")
                for k in range(2):
                    nc.tensor.matmul(ps[:],
                                     woutb_sb[k][:, ct2 * 128:(ct2 + 1) * 128],
                                     vt[k][:], start=(k == 0), stop=(k == 1))
                t = evp.tile([128, 512], f32, tag="voe")
                nc.scalar.copy(t[:], ps[:])
                nc.sync.dma_start(voutp[ct2 * 128:(ct2 + 1) * 128, s["sl"]], t[:])
            va = va_sb[w % 2]
            for ct in range(2):
                trp = pM.tile([128, 512], f32r, tag="pm")
                for jt in range(4):
                    nc.tensor.transpose(
                        trp[:, jt * 128:(jt + 1) * 128],
                        vt[ct][:, jt * 128:(jt + 1) * 128],
                        ident_sb[:])
                dst = _ap(va, 256 * ct + 32, [[512, 4], [64, 4], [1, 32]])
                nc.scalar.copy(
                    dst, trp[:].rearrange("p (a b c) -> p a b c", a=4, b=4))
            s.update(vt=vt, vpadw=vpadw, va=va)

        def emit_lepe(s):
            laccs = []
            for ct in range(2):
                la = recP.tile([128, 512], bf16, name=f"la{ct}",
                               tag=f"la{ct}", bufs=1)
                nc.gpsimd.memset(la[:], 0.0)
                laccs.append(la)
            s["lacc"] = laccs
            s["taps"] = [(ct, dy, dx) for ct in range(2) for dy in (-1, 1)
                         for dx in (-1, 0, 1)]
            s["ot_w"] = [otwP.tile([128, 512], f32r, name="otw", tag="otw")
                         for _ in range(2)]

        def emit_lepe_taps(s, i0, i1):
            for ct, dy, dx in s["taps"][i0:i1]:
                kk = (dy + 1) * 3 + (dx + 1)
                r0, nr = max(0, -dy), R - abs(dy)
                la = s["lacc"][ct]
                out_ap = _ap(la, r0 * C, [[C, nr], [1, C]])
                rhs_ap = _ap(s["vpadw"][ct], (r0 + dy) * CP + 1 + dx,
                             [[CP, nr], [1, C]])
                wcol = w18_sb[:, ct * 9 + kk:ct * 9 + kk + 1]
                nc.vector.scalar_tensor_tensor(out_ap, rhs_ap, wcol,
                                               out_ap, op0=ALU.mult,
                                               op1=ALU.add)

        def emit_heads(s, h0, h1):
            # head PAIRS with j2-interleaved score matmuls: consecutive PE
            # matmuls target different row bands so weight loads overlap.
            qt, kt, va, ot_w = s["qt"], s["kt"], s["va"], s["ot_w"]
            for hp in range(h0, h1, 2):
                pair = (hp, hp + 1)
                exs = {}
                for h in pair:
                    exs[h] = exP.tile([128, 2048], fp8, tag="ex", name=f"ex{h % 2}")
                for half in range(2):
                    sts = {}
                    for h in pair:
                        sts[h] = pST.tile([128, 1024], f32, tag="st",
                                          name=f"st{h % 2}")
                    for j2 in range(2):
                        jt = half * 2 + j2
                        for h in pair:
                            cth, hh = h // 4, h % 4
                            nc.tensor.matmul(
                                sts[h][:, j2 * 512:(j2 + 1) * 512],
                                kt[cth][32 * hh:32 * hh + 32, jt * 128:(jt + 1) * 128],
                                qt[cth][32 * hh:32 * hh + 32, :],
                                start=True, stop=True, tile_position=(32 * hh, 0))
                    for h in pair:
                        nc.scalar.activation(exs[h][:, half * 1024:(half + 1) * 1024],
                                             sts[h][:], AF.Exp)
                for h in pair:
                    cth, hh = h // 4, h % 4
                    pv = pM.tile([64, 512], f32, tag="pm")
                    for half in range(2):
                        lhs = _ap(va, 1024 * half + 64 * h, [[512, 2], [1, 64]])
                        rhs = _ap(exs[h], 1024 * half, [[512, 2], [1, 512]])
                        nc.tensor.matmul(pv[:], lhs, rhs, perf_mode=DR,
                                         start=(half == 0), stop=(half == 1))
                    rec = recP.tile([32, 512], f32, tag="rec")
                    nc.vector.reciprocal_approx_fast(rec[:], pv[0:32, :])
                    nc.vector.tensor_mul(ot_w[cth][32 * hh:32 * hh + 32, :],
                                         pv[32:64, :], rec[:])

        def emit_tail(s):
            otf = []
            for ct in range(2):
                lp = pM.tile([128, 512], f32, tag="pm")
                for i, dx in enumerate((0, -1, 1)):
                    out_ap = _ap(lp, 0, [[1, R * C]])
                    rhs_ap = _ap(s["vpadw"][ct], 1 + dx, [[CP, R], [1, C]])
                    nc.tensor.matmul(out_ap,
                                     ldiag_sb[:, (ct * 3 + dx + 1) * 128:(ct * 3 + dx + 2) * 128],
                                     rhs_ap, start=(i == 0), stop=(i == 2),
                                     skip_group_check=(i == 1))
                t = otwP.tile([128, 512], f32r, tag="otf")
                nc.vector.scalar_tensor_tensor(t[:], lp[:],
                                               cb2_sb[:, ct:ct + 1],
                                               s["ot_w"][ct][:], op0=ALU.add,
                                               op1=ALU.add)
                nc.vector.tensor_tensor(t[:], t[:], s["lacc"][ct][:],
                                        op=ALU.add)
                otf.append(t)
            for ct2 in range(2):
                ps = pM.tile([128, 512], f32, tag="pm")
                for k2 in range(2):
                    nc.tensor.matmul(ps[:],
                                     w2_sb[k2][:, ct2 * 128:(ct2 + 1) * 128],
                                     otf[k2][:], start=(k2 == 0), stop=(k2 == 1))
                t = outfP.tile([128, 512], f32, tag="outf")
                nc.vector.scalar_tensor_tensor(t[:], ps[:], b2c_sb[:, ct2:ct2 + 1],
                                               s["y0w"][ct2][:], op0=ALU.add,
                                               op1=ALU.add)
                nc.sync.dma_start(outp[ct2 * 128:(ct2 + 1) * 128, s["sl"]], t[:])

        # prologue: window 0's A-phase runs serially
        cur = emit_A1(0)
        emit_stats(cur)
        emit_x2s(cur)
        emit_qk(cur)
        emit_v(cur, 0)
        for w in range(8):
            nxt = emit_A1(w + 1) if w + 1 < 8 else None
            emit_lepe(cur)
            emit_heads(cur, 0, 2)
            emit_lepe_taps(cur, 0, 3)
            if nxt:
                emit_stats(nxt)
            emit_heads(cur, 2, 4)
            emit_lepe_taps(cur, 3, 6)
            if nxt:
                emit_x2s(nxt)
            emit_heads(cur, 4, 6)
            emit_lepe_taps(cur, 6, 9)
            if nxt:
                emit_qk(nxt)
            emit_heads(cur, 6, 8)
            emit_lepe_taps(cur, 9, 12)
            if nxt:
                emit_v(nxt, w + 1)
            emit_tail(cur)
            cur = nxt

    nc.compile()
    return nc


# ---------------------------------------------------------------------------
# host side
# ---------------------------------------------------------------------------

def _perms():
    p0 = np.arange(L).reshape(64, 8, 8).transpose(1, 0, 2).ravel()
    p1 = np.arange(L).reshape(8, 8, 64).transpose(0, 2, 1).ravel()
    return p0, p1


def _host_prep(inputs):
    x = np.asarray(inputs['x'], np.float32)
    w_embed = np.asarray(inputs['w_embed'], np.float32)
    g1 = np.asarray(inputs['g1'], np.float32)
    b1 = np.asarray(inputs['b1'], np.float32)
    w_qkv = np.asarray(inputs['w_qkv'], np.float32)
    w_proj = np.asarray(inputs['w_proj'], np.float32)
    b_proj = np.asarray(inputs['b_proj'], np.float32)
    w_out = np.asarray(inputs['w_out'], np.float32)
    conv_w = [np.asarray(inputs['conv_w0'], np.float32),
              np.asarray(inputs['conv_w1'], np.float32)]
    conv_b = [np.asarray(inputs['conv_b0'], np.float32),
              np.asarray(inputs['conv_b1'], np.float32)]
    perms = _perms()

    wemb_a = np.concatenate(
        [w_embed, w_embed.sum(1, keepdims=True) / C2], 1).astype(np.float32)
    ident = np.eye(128, dtype=np.float32)
    onesr = np.ones((1, 128), np.float32)
    onesc = np.ones((128, 1), np.float32)
    ones256 = np.ones((128, 256), np.float32)
    ones2 = np.zeros((2, 512), np.float32)
    ones2[1] = 1.0

    def fold(Wx, s):
        return g1[:, None] * Wx * s

    in_maps = []
    for c in range(8):
        b, br = c // 2, c % 2
        pm = perms[br]
        xbm = np.ascontiguousarray(x[b].reshape(256, L)[:, pm])
        o = br * 256
        wq_a = fold(w_qkv[:, o:o + 256], SCALE)
        wk_a = fold(w_qkv[:, 512 + o:512 + o + 256], 1.0)
        wv_a = fold(w_qkv[:, 1024 + o:1024 + o + 256], 1.0)
        wqkv_a = np.ascontiguousarray(np.concatenate([wq_a, wk_a, wv_a], 1))
        wb_q = (b1 @ w_qkv[:, o:o + 256]) * SCALE
        wb_k = b1 @ w_qkv[:, 512 + o:512 + o + 256]
        wb_v = b1 @ w_qkv[:, 1024 + o:1024 + o + 256]
        wbias = np.stack([wb_q[:128], wb_q[128:], wb_k[:128], wb_k[128:],
                          wb_v[:128], wb_v[128:]], 1).astype(np.float32)
        woutb = np.ascontiguousarray(w_out[o:o + 256])
        wyh = np.ascontiguousarray(w_embed[:, o:o + 256] @ woutb)
        w2 = np.ascontiguousarray(w_proj[o:o + 256] @ w_out)
        b2 = (b_proj @ w_out) if br == 0 else np.zeros(256, np.float32)
        w9 = conv_w[br].reshape(256, 3, 3)
        if br == 1:
            w9 = w9.transpose(0, 2, 1)   # transposed window image
        w9 = np.ascontiguousarray(w9.reshape(256, 9))
        ld = np.zeros((128, 6, 128), np.float32)
        for ct in range(2):
            for i, dx in enumerate((-1, 0, 1)):
                ld[np.arange(128), ct * 3 + i, np.arange(128)] = \
                    w9[ct * 128:(ct + 1) * 128, 4 + dx]
        w18 = np.ascontiguousarray(
            np.concatenate([w9[0:128], w9[128:256]], axis=1)).astype(np.float32)
        in_maps.append({
            "xb": xbm,
            "wemb": wemb_a,
            "wqkv": wqkv_a,
            "woutb": woutb,
            "wy": wyh,
            "w2": w2.astype(np.float32),
            "ldiag": ld.reshape(128, 768).astype(ml_dtypes.bfloat16),
            "w18": w18,
            "wbias": np.ascontiguousarray(wbias),
            "ident": ident,
            "onesr": onesr,
            "onesc": onesc,
            "ones256": ones256,
            "cb2": np.ascontiguousarray(conv_b[br].reshape(2, 128).T),
            "b2c": np.ascontiguousarray(b2.reshape(2, 128).T.astype(np.float32)),
        })
    return in_maps


def _gather(results):
    perms = _perms()
    out = np.zeros((B, 256, L), np.float32)
    vout = np.zeros((B, 256, L), np.float32)
    for c in range(8):
        b, br = c // 2, c % 2
        pm = perms[br]
        tmp = np.zeros((256, L), np.float32)
        tmp[:, pm] = results[c]["outp"]
        out[b] += tmp
        tmp = np.zeros((256, L), np.float32)
        tmp[:, pm] = results[c]["voutp"]
        vout[b] += tmp
    return (out.reshape(B, 256, 64, 64), vout.reshape(B, 256, 64, 64))


_CACHE = {}


def get_nc():
    if "nc" not in _CACHE:
        _CACHE["nc"] = build_nc()
    return _CACHE["nc"]


def kernel(**inputs):
    nc = get_nc()
    in_maps = _host_prep(inputs)
    res = run_bass_kernel_spmd(nc, in_maps, core_ids=list(range(8)))
    return _gather(res.results)

